# revision 13
# baseline (speedup 1.0000x reference)
"""BERT-CRF loss kernel for Trainium2 (8 NeuronCores, Bass/Tile).

Scaled-exp domain CRF forward with an augmented 34-row state per batch column:
  rows 0-31: P = exp(alpha - t*MU)   (CRF forward variables, constant drain MU)
  row 32:    omega = one-hot capture: theta_t * colsum(P_{t-1})
  row 33:    A     = running sum of captured omegas

One step (t = 1..511):  state' = (W^T state) * F_t   where
  W[:, j<32]  = exp(trans)            (CRF transition mixing)
  W[:32, 32]  = 1                     (omega' gets colsum of P)
  W[32:,  33] = 1                     (A' = A + omega)
  F_t rows 0-31 = exp(feat_t - MU), row 32 = theta_t = (len==t), row 33 = 1.

theta/ones are shipped as 2 extra host-built channels appended to feats
(pre-log: MU or -1e9), so the on-chip transpose+exp produces the full F tile
with no extra per-step ops. No masking, no renorm, no gather: after a final
virtual step 512, forward[b] = log(A + omega) + len[b]*MU. Gold score is pure
gathers, done on host. Validated: max |log| magnitude ~59 < 88 (fp32 safe).
"""

import os
import sys

import numpy as np

NUM_TAGS = 32
START = 30
STOP = 31
B = 1024
S = 512
NCORES = 8
BC = B // NCORES  # 128 batch per core
MU = 4.3
TT = NUM_TAGS + 2  # 34 channels: 32 tags + theta + one
WA = 64  # chain A (nc.vector) batch columns
WB = BC - WA  # chain B (nc.any) batch columns
NEG = -1.0e9
CH = 64  # channels padded to 64 so 2 steps fit one 128-row transpose block

for _p in ("/opt/trn_rl_repo", "/root/.axon_site/_ro/trn_rl_repo"):
    if os.path.isdir(_p) and _p not in sys.path:
        sys.path.append(_p)

_NC_CACHE = None
_LAST_RESULTS = None  # BassKernelResults of most recent device run (for test.py)


def _build_bass():
    import concourse.bacc as bacc
    import concourse.tile as tile
    from concourse import mybir

    f32 = mybir.dt.float32
    nc = bacc.Bacc(None)

    aug_d = nc.declare_dram_parameter("aug", [BC, S, CH], f32, isOutput=False)
    w_d = nc.declare_dram_parameter("w", [TT, TT], f32, isOutput=False)
    bias0_d = nc.declare_dram_parameter("bias0", [128, 1], f32, isOutput=False)
    fin_d = nc.declare_dram_parameter("fin", [TT, BC], f32, isOutput=False)
    ident_d = nc.declare_dram_parameter("ident", [128, 128], f32, isOutput=False)
    fs_d = nc.declare_dram_parameter("fs", [1, BC], f32, isOutput=True)

    Exp = mybir.ActivationFunctionType.Exp

    with tile.TileContext(nc) as tc:
        with (
            tc.tile_pool(name="const", bufs=1) as const,
            tc.tile_pool(name="fsb", bufs=3) as fsb_pool,
            tc.tile_pool(name="fexp", bufs=3) as f_pool,
            tc.tile_pool(name="pst", bufs=3) as p_pool,
            tc.tile_pool(name="tp", bufs=2, space="PSUM") as tp_pool,
            tc.tile_pool(name="q", bufs=4, space="PSUM") as q_pool,
        ):
            w_sb = const.tile([TT, TT], f32)
            nc.sync.dma_start(out=w_sb, in_=w_d[:, :])
            bias0_sb = const.tile([128, 1], f32)
            nc.sync.dma_start(out=bias0_sb, in_=bias0_d[:, :])
            ident_sb = const.tile([128, 128], f32)
            nc.sync.dma_start(out=ident_sb, in_=ident_d[:, :])
            fin_sb = const.tile([TT, BC], f32)
            ones_sb = const.tile([TT, 1], f32)
            nc.vector.memset(ones_sb, 1.0)
            biasmu_sb = const.tile([128, 1], f32)
            nc.vector.memset(biasmu_sb, -MU)
            nc.sync.dma_start(out=fin_sb, in_=fin_d[:, :])
            fs_sb = const.tile([1, BC], f32)

            # pre-touch DMA'd constants on PE so real PE ops carry <=1 new wait
            warm = q_pool.tile([TT, BC], f32, tag="q")
            nc.tensor.transpose(warm[0:34, :], ident_sb[:, 0:34], ident_sb)
            nc.tensor.matmul(warm[:, 0:TT], w_sb, w_sb, start=True, stop=True)
            nc.tensor.matmul(warm, w_sb, fin_sb, start=True, stop=True)

            pstate = None  # state [34, 128]

            def act_mul(out, a, b):
                ins = nc.any.tensor_mul(out, a, b)
                ins.engine = mybir.EngineType.Activation
                return ins

            def step(f_ap_a, f_ap_b):
                """state' = (W^T state) * F; one matmul, two elementwise muls."""
                nonlocal pstate
                q = q_pool.tile([TT, BC], f32, tag="q")
                nc.tensor.matmul(q, w_sb, pstate, start=True, stop=True)
                p_n = p_pool.tile([TT, BC], f32, tag="p")
                nc.vector.tensor_mul(p_n[:, 0:WA], q[:, 0:WA], f_ap_a)
                act_mul(p_n[:, WA:BC], q[:, WA:BC], f_ap_b)
                pstate = p_n

            for g in range(S // 8):  # 8 steps per staging group
                fsb = fsb_pool.tile([128, 8 * CH], f32, tag="fsb")
                nc.sync.dma_start(
                    out=fsb,
                    in_=aug_d[:, 8 * g : 8 * (g + 1), :].rearrange("b t j -> b (t j)"),
                )
                tp = tp_pool.tile([128, 512], f32, tag="tp")
                for k in range(4):  # 2 steps per 128x128 transpose
                    nc.tensor.transpose(
                        tp[:, 128 * k : 128 * (k + 1)],
                        fsb[:, 128 * k : 128 * (k + 1)],
                        ident_sb,
                    )
                f_tile = f_pool.tile([128, 512], f32, tag="f")
                if g == 0:
                    # col-block 0 holds t=0 (rows 0-63) whose tag rows get the
                    # START-transition bias; blocks 1-3 are regular.
                    nc.scalar.activation(
                        f_tile[:, 0:128], tp[:, 0:128], Exp,
                        bias=bias0_sb, scale=1.0,
                    )
                    nc.scalar.activation(
                        f_tile[:, 128:512], tp[:, 128:512], Exp,
                        bias=biasmu_sb, scale=1.0,
                    )
                else:
                    nc.scalar.activation(
                        f_tile, tp, Exp, bias=biasmu_sb, scale=1.0,
                    )
                for s in range(8):
                    t = 8 * g + s
                    k, par = s // 2, s % 2
                    r = 64 * par
                    c = 128 * k
                    if t == 0:
                        pstate = f_tile[0:TT, 0:128]
                        continue
                    step(
                        f_tile[r : r + TT, c : c + WA],
                        f_tile[r : r + TT, c + WA : c + BC],
                    )

            # virtual step 512: capture len==512 columns via fin
            step(fin_sb[:, 0:WA], fin_sb[:, WA:BC])

            # fs = A + omega = ones^T state[32:34]  (32-aligned partition base)
            fs_ps = q_pool.tile([1, BC], f32, tag="fsps", bufs=1)
            nc.tensor.matmul(
                fs_ps, ones_sb[32:34, :], pstate[32:34, :], start=True, stop=True
            )
            nc.vector.tensor_copy(fs_sb, fs_ps)
            nc.sync.dma_start(out=fs_d[:, :], in_=fs_sb)

    if not nc.is_finalized():
        nc.finalize()
    return nc


def _gold_score(feats, labels, lengths, trans):
    pos = np.arange(S)[None, :]
    valid = pos < lengths[:, None]
    emit = np.take_along_axis(feats, labels[:, :, None], axis=2)[:, :, 0]
    emit_sum = np.where(valid, emit, 0.0).sum(axis=1)
    start_sc = trans[START, labels[:, 0]]
    pair = trans[labels[:, :-1], labels[:, 1:]]
    pair_sum = np.where(valid[:, 1:], pair, 0.0).sum(axis=1)
    last_tag = np.take_along_axis(labels, (lengths - 1)[:, None], axis=1)[:, 0]
    stop_sc = trans[last_tag, STOP]
    return emit_sum + start_sc + pair_sum + stop_sc


def kernel(feats, labels, lengths, transitions):
    global _NC_CACHE, _LAST_RESULTS
    from concourse.bass_utils import run_bass_kernel_spmd

    feats = np.asarray(feats, dtype=np.float32)
    labels = np.asarray(labels, dtype=np.int64)
    lengths = np.asarray(lengths, dtype=np.int64)
    trans = np.asarray(transitions, dtype=np.float32)

    if _NC_CACHE is None:
        _NC_CACHE = _build_bass()
    nc = _NC_CACHE

    # augmented feats: [B, S, 64] = [feats | theta_log | ones_log | pad]
    aug = np.zeros((B, S, CH), np.float32)
    aug[:, :, :NUM_TAGS] = feats
    aug[:, :, 32] = NEG
    aug[np.arange(B), np.minimum(lengths, S - 1), 32] = np.where(
        lengths <= S - 1, MU, NEG
    )  # theta_t one-hot at t == len (t<=511 handled here; len==512 via fin)
    aug[:, :, 33] = MU
    aug[:, 0, 33] = NEG  # A_0 = 0

    # weight matrix [34, 34]
    w = np.zeros((TT, TT), np.float32)
    w[:NUM_TAGS, :NUM_TAGS] = np.exp(trans)
    w[:NUM_TAGS, 32] = 1.0  # omega' = theta * colsum(P)
    w[32, 33] = 1.0  # A' = A + omega
    w[33, 33] = 1.0

    # bias for the first transpose block (t=0 rows 0-31 add trans[START])
    bias0 = np.full((128, 1), -MU, np.float32)
    bias0[0:NUM_TAGS, 0] = trans[START, :] - MU

    ident = np.eye(128, dtype=np.float32)

    # virtual step 512 multiplier: theta = (len==512), ones-row = 1, P rows = 0
    fin_full = np.zeros((TT, B), np.float32)
    fin_full[32] = (lengths == S).astype(np.float32)
    fin_full[33] = 1.0

    in_maps = []
    for c in range(NCORES):
        sl = slice(c * BC, (c + 1) * BC)
        in_maps.append(
            {
                "aug": aug[sl],
                "w": w,
                "bias0": bias0,
                "fin": np.ascontiguousarray(fin_full[:, sl]),
                "ident": ident,
            }
        )

    trace = bool(int(os.environ.get("BASS_KERNEL_TRACE", "0")))
    res = run_bass_kernel_spmd(nc, in_maps, list(range(NCORES)), trace=trace)
    _LAST_RESULTS = res

    fs = np.concatenate([res.results[c]["fs"][0] for c in range(NCORES)])  # [B]
    forward = np.log(fs.astype(np.float64)) + lengths * MU
    gold = _gold_score(feats, labels, lengths, trans).astype(np.float64)
    loss = np.sum(forward - gold) / B
    return np.asarray(loss, dtype=np.float32)


# revision 15
# speedup vs baseline: 3362.2312x; 3362.2312x over previous
"""BERT-CRF loss kernel for Trainium2 (8 NeuronCores, Bass/Tile).

Scaled-exp domain CRF forward with an augmented 34-row state per batch column:
  rows 0-31: P = exp(alpha - t*MU)   (CRF forward variables, constant drain MU)
  row 32:    omega = one-hot capture: theta_t * colsum(P_{t-1})
  row 33:    A     = running sum of captured omegas

One step (t = 1..511):  state' = (W^T state) * F_t   where
  W[:, j<32]  = exp(trans)            (CRF transition mixing)
  W[:32, 32]  = 1                     (omega' gets colsum of P)
  W[32:,  33] = 1                     (A' = A + omega)
  F_t rows 0-31 = exp(feat_t - MU), row 32 = theta_t = (len==t), row 33 = 1.

theta/ones are shipped as 2 extra host-built channels appended to feats
(pre-log: MU or -1e9), so the on-chip transpose+exp produces the full F tile
with no extra per-step ops. No masking, no renorm, no gather: after a final
virtual step 512, forward[b] = log(A + omega) + len[b]*MU. Gold score is pure
gathers, done on host. Validated: max |log| magnitude ~59 < 88 (fp32 safe).
"""

import os
import sys

import numpy as np

NUM_TAGS = 32
START = 30
STOP = 31
B = 1024
S = 512
NCORES = 8
BC = B // NCORES  # 128 batch per core
MU = 4.3
TT = NUM_TAGS + 2  # 34 channels: 32 tags + theta + one
WA = 64  # chain A (nc.vector) batch columns
WB = BC - WA  # chain B (nc.any) batch columns
NEG = -1.0e9
CH = 64  # channels padded to 64 so 2 steps fit one 128-row transpose block

for _p in ("/opt/trn_rl_repo", "/root/.axon_site/_ro/trn_rl_repo"):
    if os.path.isdir(_p) and _p not in sys.path:
        sys.path.append(_p)

_NC_CACHE = None
_LAST_RESULTS = None  # BassKernelResults of most recent device run (for test.py)


def _install_ntff_hook():
    """Shim antenv.axon_hooks (absent in this image) so trace=True works."""
    import types

    if "antenv.axon_hooks" in sys.modules:
        return
    mod = types.ModuleType("antenv.axon_hooks")
    mod._hook = None
    mod.set_axon_ntff_profile_hook = lambda h: setattr(mod, "_hook", h)
    mod.get_axon_ntff_profile_hook = lambda: mod._hook
    sys.modules["antenv.axon_hooks"] = mod
    try:
        import antenv

        antenv.axon_hooks = mod
    except ImportError:
        pass
    try:
        from trn_agent_boot.trn_boot import _ntff_profile_via_ctypes

        h = _ntff_profile_via_ctypes("/opt/axon/libaxon_pjrt.so")
        if h is not None:
            mod._hook = h
    except Exception:
        pass


def _build_bass():
    import concourse.bacc as bacc
    import concourse.tile as tile
    from concourse import mybir

    f32 = mybir.dt.float32
    nc = bacc.Bacc(None)

    aug_d = nc.declare_dram_parameter("aug", [BC, S, CH], f32, isOutput=False)
    w_d = nc.declare_dram_parameter("w", [TT, TT], f32, isOutput=False)
    bias0_d = nc.declare_dram_parameter("bias0", [128, 1], f32, isOutput=False)
    fin_d = nc.declare_dram_parameter("fin", [TT, BC], f32, isOutput=False)
    ident_d = nc.declare_dram_parameter("ident", [128, 128], f32, isOutput=False)
    fs_d = nc.declare_dram_parameter("fs", [1, BC], f32, isOutput=True)

    Exp = mybir.ActivationFunctionType.Exp

    with tile.TileContext(nc) as tc:
        with (
            tc.tile_pool(name="const", bufs=1) as const,
            tc.tile_pool(name="fsb", bufs=3) as fsb_pool,
            tc.tile_pool(name="fexp", bufs=3) as f_pool,
            tc.tile_pool(name="pst", bufs=3) as p_pool,
            tc.tile_pool(name="tp", bufs=2, space="PSUM") as tp_pool,
            tc.tile_pool(name="q", bufs=4, space="PSUM") as q_pool,
        ):
            w_sb = const.tile([TT, TT], f32)
            nc.sync.dma_start(out=w_sb, in_=w_d[:, :])
            bias0_sb = const.tile([128, 1], f32)
            nc.sync.dma_start(out=bias0_sb, in_=bias0_d[:, :])
            ident_sb = const.tile([128, 128], f32)
            nc.sync.dma_start(out=ident_sb, in_=ident_d[:, :])
            fin_sb = const.tile([TT, BC], f32)
            ones_sb = const.tile([TT, 1], f32)
            nc.vector.memset(ones_sb, 1.0)
            biasmu_sb = const.tile([128, 1], f32)
            nc.vector.memset(biasmu_sb, -MU)
            nc.sync.dma_start(out=fin_sb, in_=fin_d[:, :])
            fs_sb = const.tile([1, BC], f32)

            # pre-touch DMA'd constants on PE so real PE ops carry <=1 new wait
            warm = q_pool.tile([TT, BC], f32, tag="q")
            nc.tensor.transpose(warm[0:34, :], ident_sb[:, 0:34], ident_sb)
            nc.tensor.matmul(warm[:, 0:TT], w_sb, w_sb, start=True, stop=True)
            nc.tensor.matmul(warm, w_sb, fin_sb, start=True, stop=True)

            pstate = None  # state [34, 128]

            def act_mul(out, a, b):
                ins = nc.any.tensor_mul(out, a, b)
                ins.engine = mybir.EngineType.Activation
                return ins

            def step(f_ap_a, f_ap_b):
                """state' = (W^T state) * F; one matmul, two elementwise muls."""
                nonlocal pstate
                q = q_pool.tile([TT, BC], f32, tag="q")
                nc.tensor.matmul(q, w_sb, pstate, start=True, stop=True)
                p_n = p_pool.tile([TT, BC], f32, tag="p")
                nc.vector.tensor_mul(p_n[:, 0:WA], q[:, 0:WA], f_ap_a)
                act_mul(p_n[:, WA:BC], q[:, WA:BC], f_ap_b)
                pstate = p_n

            for g in range(S // 8):  # 8 steps per staging group
                fsb = fsb_pool.tile([128, 8 * CH], f32, tag="fsb")
                nc.sync.dma_start(
                    out=fsb,
                    in_=aug_d[:, 8 * g : 8 * (g + 1), :].rearrange("b t j -> b (t j)"),
                )
                tp = tp_pool.tile([128, 512], f32, tag="tp")
                for k in range(4):  # 2 steps per 128x128 transpose
                    nc.tensor.transpose(
                        tp[:, 128 * k : 128 * (k + 1)],
                        fsb[:, 128 * k : 128 * (k + 1)],
                        ident_sb,
                    )
                f_tile = f_pool.tile([128, 512], f32, tag="f")
                if g == 0:
                    # col-block 0 holds t=0 (rows 0-63) whose tag rows get the
                    # START-transition bias; blocks 1-3 are regular.
                    nc.scalar.activation(
                        f_tile[:, 0:128], tp[:, 0:128], Exp,
                        bias=bias0_sb, scale=1.0,
                    )
                    nc.scalar.activation(
                        f_tile[:, 128:512], tp[:, 128:512], Exp,
                        bias=biasmu_sb, scale=1.0,
                    )
                else:
                    nc.scalar.activation(
                        f_tile, tp, Exp, bias=biasmu_sb, scale=1.0,
                    )
                for s in range(8):
                    t = 8 * g + s
                    k, par = s // 2, s % 2
                    r = 64 * par
                    c = 128 * k
                    if t == 0:
                        pstate = f_tile[0:TT, 0:128]
                        continue
                    step(
                        f_tile[r : r + TT, c : c + WA],
                        f_tile[r : r + TT, c + WA : c + BC],
                    )

            # virtual step 512: capture len==512 columns via fin
            step(fin_sb[:, 0:WA], fin_sb[:, WA:BC])

            # fs = A + omega = ones^T state[32:34]  (32-aligned partition base)
            fs_ps = q_pool.tile([1, BC], f32, tag="fsps", bufs=1)
            nc.tensor.matmul(
                fs_ps, ones_sb[32:34, :], pstate[32:34, :], start=True, stop=True
            )
            nc.vector.tensor_copy(fs_sb, fs_ps)
            nc.sync.dma_start(out=fs_d[:, :], in_=fs_sb)

    if not nc.is_finalized():
        nc.finalize()
    return nc


def _gold_score(feats, labels, lengths, trans):
    pos = np.arange(S)[None, :]
    valid = pos < lengths[:, None]
    emit = np.take_along_axis(feats, labels[:, :, None], axis=2)[:, :, 0]
    emit_sum = np.where(valid, emit, 0.0).sum(axis=1)
    start_sc = trans[START, labels[:, 0]]
    pair = trans[labels[:, :-1], labels[:, 1:]]
    pair_sum = np.where(valid[:, 1:], pair, 0.0).sum(axis=1)
    last_tag = np.take_along_axis(labels, (lengths - 1)[:, None], axis=1)[:, 0]
    stop_sc = trans[last_tag, STOP]
    return emit_sum + start_sc + pair_sum + stop_sc


def kernel(feats, labels, lengths, transitions):
    global _NC_CACHE, _LAST_RESULTS
    from concourse.bass_utils import run_bass_kernel_spmd

    feats = np.asarray(feats, dtype=np.float32)
    labels = np.asarray(labels, dtype=np.int64)
    lengths = np.asarray(lengths, dtype=np.int64)
    trans = np.asarray(transitions, dtype=np.float32)

    if _NC_CACHE is None:
        _NC_CACHE = _build_bass()
    nc = _NC_CACHE

    # augmented feats: [B, S, 64] = [feats | theta_log | ones_log | pad]
    aug = np.zeros((B, S, CH), np.float32)
    aug[:, :, :NUM_TAGS] = feats
    aug[:, :, 32] = NEG
    aug[np.arange(B), np.minimum(lengths, S - 1), 32] = np.where(
        lengths <= S - 1, MU, NEG
    )  # theta_t one-hot at t == len (t<=511 handled here; len==512 via fin)
    aug[:, :, 33] = MU
    aug[:, 0, 33] = NEG  # A_0 = 0

    # weight matrix [34, 34]
    w = np.zeros((TT, TT), np.float32)
    w[:NUM_TAGS, :NUM_TAGS] = np.exp(trans)
    w[:NUM_TAGS, 32] = 1.0  # omega' = theta * colsum(P)
    w[32, 33] = 1.0  # A' = A + omega
    w[33, 33] = 1.0

    # bias for the first transpose block (t=0 rows 0-31 add trans[START])
    bias0 = np.full((128, 1), -MU, np.float32)
    bias0[0:NUM_TAGS, 0] = trans[START, :] - MU

    ident = np.eye(128, dtype=np.float32)

    # virtual step 512 multiplier: theta = (len==512), ones-row = 1, P rows = 0
    fin_full = np.zeros((TT, B), np.float32)
    fin_full[32] = (lengths == S).astype(np.float32)
    fin_full[33] = 1.0

    in_maps = []
    for c in range(NCORES):
        sl = slice(c * BC, (c + 1) * BC)
        in_maps.append(
            {
                "aug": aug[sl],
                "w": w,
                "bias0": bias0,
                "fin": np.ascontiguousarray(fin_full[:, sl]),
                "ident": ident,
            }
        )

    trace = bool(int(os.environ.get("BASS_KERNEL_TRACE", "0")))
    kw = {}
    if trace:
        import concourse.bass_utils as _bu

        _install_ntff_hook()
        _bu.upload_artifacts = lambda tmpdir: "local://" + tmpdir
        import tempfile

        root = os.environ.get("BASS_TRACE_DIR", "/tmp/bass_trace")
        os.makedirs(root, exist_ok=True)
        tdir = tempfile.mkdtemp(dir=root)
        kw = {"tmpdir": tdir}
    res = run_bass_kernel_spmd(nc, in_maps, list(range(NCORES)), trace=trace, **kw)
    _LAST_RESULTS = res

    fs = np.concatenate([res.results[c]["fs"][0] for c in range(NCORES)])  # [B]
    forward = np.log(fs.astype(np.float64)) + lengths * MU
    gold = _gold_score(feats, labels, lengths, trans).astype(np.float64)
    loss = np.sum(forward - gold) / B
    return np.asarray(loss, dtype=np.float32)


# revision 17
# speedup vs baseline: 5249.7098x; 1.5614x over previous
"""BERT-CRF loss kernel for Trainium2 (8 NeuronCores, Bass/Tile).

Scaled-exp domain CRF forward with an exactly-32-row state per batch column:
  rows 0-30: P = exp(alpha[kept_tags] - t*MU)   (START tag dropped: provably 0)
  row 31:    omega = captured forward value (self-holding)

One step (t = 1..511):  state' = (W^T state) * F_t   where
  W[:31, :31] = exp(trans)[kept, kept]   (CRF transition mixing)
  W[:31, 31]  = 1, W[31, 31] = 1         (omega' = colsum(P) + omega)
  F_t rows 0-30 = exp(feat_t - MU)  (0 at the death step t==len: host scatter)
  F_t row 31    = theta_t = (t >= len)   (step fn; omega captures once because
                                          P dies at the death step, so colsum
                                          contributes only at t==len)

K=M=32 keeps every matmul in a single PE row/col group (one instruction), and
the 32-channel augmented feats keep all SBUF partition slices 32-aligned.
After a final virtual step 512 (captures len==512), forward[b] =
log(omega) + len[b]*MU. Gold score is pure gathers, done on host.
Validated: max |log| magnitude ~59 < 88 (fp32 safe) with MU=4.3.
"""

import os
import sys

import numpy as np

NUM_TAGS = 32
START = 30
STOP = 31
B = 1024
S = 512
NCORES = 8
BC = B // NCORES  # 128 batch per core
MU = 4.3
ST = 32  # state rows: 31 kept tags + omega
WA = 64  # chain A (DVE) batch columns
WB = BC - WA  # chain B (ACT) batch columns
NEG = -1.0e9
KEPT = list(range(30)) + [31]  # all tags except START

for _p in ("/opt/trn_rl_repo", "/root/.axon_site/_ro/trn_rl_repo"):
    if os.path.isdir(_p) and _p not in sys.path:
        sys.path.append(_p)

_NC_CACHE = None
_LAST_RESULTS = None  # BassKernelResults of most recent device run (for test.py)


def _install_ntff_hook():
    """Shim antenv.axon_hooks (absent in this image) so trace=True works."""
    import types

    if "antenv.axon_hooks" in sys.modules:
        return
    mod = types.ModuleType("antenv.axon_hooks")
    mod._hook = None
    mod.set_axon_ntff_profile_hook = lambda h: setattr(mod, "_hook", h)
    mod.get_axon_ntff_profile_hook = lambda: mod._hook
    sys.modules["antenv.axon_hooks"] = mod
    try:
        import antenv

        antenv.axon_hooks = mod
    except ImportError:
        pass
    try:
        from trn_agent_boot.trn_boot import _ntff_profile_via_ctypes

        h = _ntff_profile_via_ctypes("/opt/axon/libaxon_pjrt.so")
        if h is not None:
            mod._hook = h
    except Exception:
        pass


def _build_bass():
    import concourse.bacc as bacc
    import concourse.tile as tile
    from concourse import mybir

    f32 = mybir.dt.float32
    nc = bacc.Bacc(None)

    aug_d = nc.declare_dram_parameter("aug", [BC, S, ST], f32, isOutput=False)
    w_d = nc.declare_dram_parameter("w", [ST, ST], f32, isOutput=False)
    bias0_d = nc.declare_dram_parameter("bias0", [128, 1], f32, isOutput=False)
    fin_d = nc.declare_dram_parameter("fin", [ST, BC], f32, isOutput=False)
    esel_d = nc.declare_dram_parameter("esel", [ST, 1], f32, isOutput=False)
    ident_d = nc.declare_dram_parameter("ident", [128, 128], f32, isOutput=False)
    fs_d = nc.declare_dram_parameter("fs", [1, BC], f32, isOutput=True)

    Exp = mybir.ActivationFunctionType.Exp

    with tile.TileContext(nc) as tc:
        with (
            tc.tile_pool(name="const", bufs=1) as const,
            tc.tile_pool(name="fsb", bufs=3) as fsb_pool,
            tc.tile_pool(name="fexp", bufs=3) as f_pool,
            tc.tile_pool(name="pa", bufs=2) as pa_pool,
            tc.tile_pool(name="pb", bufs=2) as pb_pool,
            tc.tile_pool(name="tp", bufs=2, space="PSUM") as tp_pool,
            tc.tile_pool(name="qa", bufs=2, space="PSUM") as qa_pool,
            tc.tile_pool(name="qb", bufs=2, space="PSUM") as qb_pool,
        ):
            w_sb = const.tile([ST, ST], f32)
            nc.sync.dma_start(out=w_sb, in_=w_d[:, :])
            bias0_sb = const.tile([128, 1], f32)
            nc.sync.dma_start(out=bias0_sb, in_=bias0_d[:, :])
            ident_sb = const.tile([128, 128], f32)
            nc.sync.dma_start(out=ident_sb, in_=ident_d[:, :])
            fin_sb = const.tile([ST, BC], f32)
            nc.sync.dma_start(out=fin_sb, in_=fin_d[:, :])
            esel_sb = const.tile([ST, 1], f32)
            nc.sync.dma_start(out=esel_sb, in_=esel_d[:, :])
            biasmu_sb = const.tile([128, 1], f32)
            nc.vector.memset(biasmu_sb, -MU)
            fs_sb = const.tile([1, BC], f32)

            # pre-touch DMA'd constants on PE so real PE ops carry <=1 new wait
            warm = qa_pool.tile([ST, BC], f32, tag="qa")
            nc.tensor.transpose(warm[0:ST, :], ident_sb[:, 0:ST], ident_sb)
            nc.tensor.matmul(warm[:, 0:ST], w_sb, w_sb, start=True, stop=True)
            nc.tensor.matmul(warm, w_sb, fin_sb, start=True, stop=True)
            nc.tensor.matmul(warm[:, 0:1], w_sb, esel_sb, start=True, stop=True)

            pa = None  # chain A state [32, WA]
            pb = None  # chain B state [32, WB]

            def step(f_ap_a, f_ap_b):
                """state' = (W^T state) * F; per chain: matmul + elementwise."""
                nonlocal pa, pb
                qa = qa_pool.tile([ST, WA], f32, tag="qa")
                nc.tensor.matmul(qa, w_sb, pa, start=True, stop=True)
                pa_n = pa_pool.tile([ST, WA], f32, tag="pa")
                nc.vector.tensor_mul(pa_n, qa, f_ap_a)
                qb = qb_pool.tile([ST, WB], f32, tag="qb")
                nc.tensor.matmul(qb, w_sb, pb, start=True, stop=True)
                pb_n = pb_pool.tile([ST, WB], f32, tag="pb")
                nc.vector.tensor_mul(pb_n, qb, f_ap_b)
                pa, pb = pa_n, pb_n

            for g in range(S // 16):  # 16 steps per staging group
                fsb = fsb_pool.tile([128, 16 * ST], f32, tag="fsb")
                nc.sync.dma_start(
                    out=fsb,
                    in_=aug_d[:, 16 * g : 16 * (g + 1), :].rearrange(
                        "b t j -> b (t j)"
                    ),
                )
                tp = tp_pool.tile([128, 512], f32, tag="tp")
                for k in range(4):  # 4 steps per 128x128 transpose
                    nc.tensor.transpose(
                        tp[:, 128 * k : 128 * (k + 1)],
                        fsb[:, 128 * k : 128 * (k + 1)],
                        ident_sb,
                    )
                f_tile = f_pool.tile([128, 512], f32, tag="f")
                if g == 0:
                    # col-block 0 holds t=0..3; its partition rows 0-31 are
                    # t=0, which get the START-transition bias.
                    nc.scalar.activation(
                        f_tile[:, 0:128], tp[:, 0:128], Exp,
                        bias=bias0_sb, scale=1.0,
                    )
                    nc.scalar.activation(
                        f_tile[:, 128:512], tp[:, 128:512], Exp,
                        bias=biasmu_sb, scale=1.0,
                    )
                else:
                    nc.scalar.activation(
                        f_tile, tp, Exp, bias=biasmu_sb, scale=1.0,
                    )
                for s in range(16):
                    t = 16 * g + s
                    k, sub = s // 4, s % 4
                    r = 32 * sub
                    c = 128 * k
                    if t == 0:
                        pa = f_tile[0:ST, 0:WA]
                        pb = f_tile[0:ST, WA:BC]
                        continue
                    step(
                        f_tile[r : r + ST, c : c + WA],
                        f_tile[r : r + ST, c + WA : c + BC],
                    )

            # virtual step 512: capture len==512 columns via fin
            step(fin_sb[:, 0:WA], fin_sb[:, WA:BC])

            # fs = omega row = esel^T state
            fs_ps = qa_pool.tile([1, BC], f32, tag="fsps", bufs=1)
            nc.tensor.matmul(fs_ps[:, 0:WA], esel_sb, pa, start=True, stop=True)
            nc.tensor.matmul(fs_ps[:, WA:BC], esel_sb, pb, start=True, stop=True)
            nc.vector.tensor_copy(fs_sb, fs_ps)
            nc.sync.dma_start(out=fs_d[:, :], in_=fs_sb)

    if not nc.is_finalized():
        nc.finalize()
    return nc


def _gold_score(feats, labels, lengths, trans):
    pos = np.arange(S)[None, :]
    valid = pos < lengths[:, None]
    emit = np.take_along_axis(feats, labels[:, :, None], axis=2)[:, :, 0]
    emit_sum = np.where(valid, emit, 0.0).sum(axis=1)
    start_sc = trans[START, labels[:, 0]]
    pair = trans[labels[:, :-1], labels[:, 1:]]
    pair_sum = np.where(valid[:, 1:], pair, 0.0).sum(axis=1)
    last_tag = np.take_along_axis(labels, (lengths - 1)[:, None], axis=1)[:, 0]
    stop_sc = trans[last_tag, STOP]
    return emit_sum + start_sc + pair_sum + stop_sc


def kernel(feats, labels, lengths, transitions):
    global _NC_CACHE, _LAST_RESULTS
    from concourse.bass_utils import run_bass_kernel_spmd

    feats = np.asarray(feats, dtype=np.float32)
    labels = np.asarray(labels, dtype=np.int64)
    lengths = np.asarray(lengths, dtype=np.int64)
    trans = np.asarray(transitions, dtype=np.float32)

    if _NC_CACHE is None:
        _NC_CACHE = _build_bass()
    nc = _NC_CACHE

    # augmented feats: [B, S, 32] = [feats[kept] | theta_log]
    aug = np.empty((B, S, ST), np.float32)
    aug[:, :, :31] = feats[:, :, KEPT]
    rows = lengths <= S - 1
    aug[np.arange(B)[rows], lengths[rows], :31] = NEG  # kill column at t==len
    aug[:, :, 31] = np.where(
        np.arange(S)[None, :] >= lengths[:, None], MU, NEG
    )  # theta step fn (0 at t=0 since len>=1)

    w = np.zeros((ST, ST), np.float32)
    w[:31, :31] = np.exp(trans)[np.ix_(KEPT, KEPT)]
    w[:31, 31] = 1.0  # omega' = colsum(P) + omega
    w[31, 31] = 1.0

    bias0 = np.full((128, 1), -MU, np.float32)
    bias0[0:31, 0] = trans[START, KEPT] - MU

    ident = np.eye(128, dtype=np.float32)

    fin_full = np.zeros((ST, B), np.float32)
    fin_full[31] = 1.0  # virtual step 512: theta=1 everywhere, tags die

    esel = np.zeros((ST, 1), np.float32)
    esel[31, 0] = 1.0

    in_maps = []
    for c in range(NCORES):
        sl = slice(c * BC, (c + 1) * BC)
        in_maps.append(
            {
                "aug": aug[sl],
                "w": w,
                "bias0": bias0,
                "fin": np.ascontiguousarray(fin_full[:, sl]),
                "esel": esel,
                "ident": ident,
            }
        )

    trace = bool(int(os.environ.get("BASS_KERNEL_TRACE", "0")))
    kw = {}
    if trace:
        import concourse.bass_utils as _bu

        _install_ntff_hook()
        _bu.upload_artifacts = lambda tmpdir: "local://" + tmpdir
        import tempfile

        root = os.environ.get("BASS_TRACE_DIR", "/tmp/bass_trace")
        os.makedirs(root, exist_ok=True)
        tdir = tempfile.mkdtemp(dir=root)
        kw = {"tmpdir": tdir}
    res = run_bass_kernel_spmd(nc, in_maps, list(range(NCORES)), trace=trace, **kw)
    _LAST_RESULTS = res

    fs = np.concatenate([res.results[c]["fs"][0] for c in range(NCORES)])  # [B]
    forward = np.log(fs.astype(np.float64)) + lengths * MU
    gold = _gold_score(feats, labels, lengths, trans).astype(np.float64)
    loss = np.sum(forward - gold) / B
    return np.asarray(loss, dtype=np.float32)


# revision 18
# speedup vs baseline: 5957.4331x; 1.1348x over previous
"""BERT-CRF loss kernel for Trainium2 (8 NeuronCores, Bass/Tile).

Scaled-exp domain CRF forward with an exactly-32-row state per batch column:
  rows 0-30: P = exp(alpha[kept_tags] - t*MU)   (START tag dropped: provably 0)
  row 31:    omega = captured forward value (self-holding)

One step (t = 1..511):  state' = (W^T state) * F_t   where
  W[:31, :31] = exp(trans)[kept, kept]   (CRF transition mixing)
  W[:31, 31]  = 1, W[31, 31] = 1         (omega' = colsum(P) + omega)
  F_t rows 0-30 = exp(feat_t - MU)  (0 at the death step t==len: host scatter)
  F_t row 31    = theta_t = (t >= len)   (step fn; omega captures once because
                                          P dies at the death step, so colsum
                                          contributes only at t==len)

K=M=32 keeps every matmul in a single PE row/col group (one instruction), and
the 32-channel augmented feats keep all SBUF partition slices 32-aligned.
After a final virtual step 512 (captures len==512), forward[b] =
log(omega) + len[b]*MU. Gold score is pure gathers, done on host.
Validated: max |log| magnitude ~59 < 88 (fp32 safe) with MU=4.3.
"""

import os
import sys

import numpy as np

NUM_TAGS = 32
START = 30
STOP = 31
B = 1024
S = 512
NCORES = 8
BC = B // NCORES  # 128 batch per core
MU = 4.3
ST = 32  # state rows: 31 kept tags + omega
WA = 64  # chain A (DVE) batch columns
WB = BC - WA  # chain B (ACT) batch columns
NEG = -1.0e9
KEPT = list(range(30)) + [31]  # all tags except START

for _p in ("/opt/trn_rl_repo", "/root/.axon_site/_ro/trn_rl_repo"):
    if os.path.isdir(_p) and _p not in sys.path:
        sys.path.append(_p)

_NC_CACHE = None
_LAST_RESULTS = None  # BassKernelResults of most recent device run (for test.py)


def _install_ntff_hook():
    """Shim antenv.axon_hooks (absent in this image) so trace=True works."""
    import types

    if "antenv.axon_hooks" in sys.modules:
        return
    mod = types.ModuleType("antenv.axon_hooks")
    mod._hook = None
    mod.set_axon_ntff_profile_hook = lambda h: setattr(mod, "_hook", h)
    mod.get_axon_ntff_profile_hook = lambda: mod._hook
    sys.modules["antenv.axon_hooks"] = mod
    try:
        import antenv

        antenv.axon_hooks = mod
    except ImportError:
        pass
    try:
        from trn_agent_boot.trn_boot import _ntff_profile_via_ctypes

        h = _ntff_profile_via_ctypes("/opt/axon/libaxon_pjrt.so")
        if h is not None:
            mod._hook = h
    except Exception:
        pass


def _build_bass():
    import concourse.bacc as bacc
    import concourse.tile as tile
    from concourse import mybir

    f32 = mybir.dt.float32
    bf16 = mybir.dt.bfloat16
    nc = bacc.Bacc(None)

    aug_d = nc.declare_dram_parameter("aug", [BC, S, ST], f32, isOutput=False)
    w_d = nc.declare_dram_parameter("w", [ST, ST], bf16, isOutput=False)
    bias0_d = nc.declare_dram_parameter("bias0", [128, 1], f32, isOutput=False)
    fin_d = nc.declare_dram_parameter("fin", [ST, BC], bf16, isOutput=False)
    esel_d = nc.declare_dram_parameter("esel", [ST, 1], bf16, isOutput=False)
    ident_d = nc.declare_dram_parameter("ident", [128, 128], f32, isOutput=False)
    fs_d = nc.declare_dram_parameter("fs", [1, BC], f32, isOutput=True)

    Exp = mybir.ActivationFunctionType.Exp

    with tile.TileContext(nc) as tc:
        with (
            tc.tile_pool(name="const", bufs=1) as const,
            tc.tile_pool(name="fsb", bufs=3) as fsb_pool,
            tc.tile_pool(name="fexp", bufs=3) as f_pool,
            tc.tile_pool(name="pa", bufs=2) as pa_pool,
            tc.tile_pool(name="pb", bufs=2) as pb_pool,
            tc.tile_pool(name="tp", bufs=2, space="PSUM") as tp_pool,
            tc.tile_pool(name="qa", bufs=2, space="PSUM") as qa_pool,
            tc.tile_pool(name="qb", bufs=2, space="PSUM") as qb_pool,
        ):
            w_sb = const.tile([ST, ST], bf16)
            nc.sync.dma_start(out=w_sb, in_=w_d[:, :])
            bias0_sb = const.tile([128, 1], f32)
            nc.sync.dma_start(out=bias0_sb, in_=bias0_d[:, :])
            ident_sb = const.tile([128, 128], f32)
            nc.sync.dma_start(out=ident_sb, in_=ident_d[:, :])
            fin_sb = const.tile([ST, BC], bf16)
            nc.sync.dma_start(out=fin_sb, in_=fin_d[:, :])
            esel_sb = const.tile([ST, 1], bf16)
            nc.sync.dma_start(out=esel_sb, in_=esel_d[:, :])
            biasmu_sb = const.tile([128, 1], f32)
            nc.vector.memset(biasmu_sb, -MU)
            fs_sb = const.tile([1, BC], f32)

            # pre-touch DMA'd constants on PE so real PE ops carry <=1 new wait
            warm = qa_pool.tile([ST, BC], f32, tag="qa")
            nc.tensor.transpose(warm[0:ST, :], ident_sb[:, 0:ST], ident_sb)
            nc.tensor.matmul(warm[:, 0:ST], w_sb, w_sb, start=True, stop=True)
            nc.tensor.matmul(warm, w_sb, fin_sb, start=True, stop=True)
            nc.tensor.matmul(warm[:, 0:1], w_sb, esel_sb, start=True, stop=True)

            pa = None  # chain A state [32, WA]
            pb = None  # chain B state [32, WB]

            def step(f_ap_a, f_ap_b):
                """state' = (W^T state) * F; per chain: matmul + elementwise."""
                nonlocal pa, pb
                qa = qa_pool.tile([ST, WA], f32, tag="qa")
                nc.tensor.matmul(qa, w_sb, pa, start=True, stop=True)
                pa_n = pa_pool.tile([ST, WA], bf16, tag="pa")
                nc.vector.tensor_mul(pa_n, qa, f_ap_a)
                qb = qb_pool.tile([ST, WB], f32, tag="qb")
                nc.tensor.matmul(qb, w_sb, pb, start=True, stop=True)
                pb_n = pb_pool.tile([ST, WB], bf16, tag="pb")
                nc.vector.tensor_mul(pb_n, qb, f_ap_b)
                pa, pb = pa_n, pb_n

            for g in range(S // 16):  # 16 steps per staging group
                fsb = fsb_pool.tile([128, 16 * ST], f32, tag="fsb")
                nc.sync.dma_start(
                    out=fsb,
                    in_=aug_d[:, 16 * g : 16 * (g + 1), :].rearrange(
                        "b t j -> b (t j)"
                    ),
                )
                tp = tp_pool.tile([128, 512], f32, tag="tp")
                for k in range(4):  # 4 steps per 128x128 transpose
                    nc.tensor.transpose(
                        tp[:, 128 * k : 128 * (k + 1)],
                        fsb[:, 128 * k : 128 * (k + 1)],
                        ident_sb,
                    )
                f_tile = f_pool.tile([128, 512], bf16, tag="f")
                if g == 0:
                    # col-block 0 holds t=0..3; its partition rows 0-31 are
                    # t=0, which get the START-transition bias.
                    nc.scalar.activation(
                        f_tile[:, 0:128], tp[:, 0:128], Exp,
                        bias=bias0_sb, scale=1.0,
                    )
                    nc.scalar.activation(
                        f_tile[:, 128:512], tp[:, 128:512], Exp,
                        bias=biasmu_sb, scale=1.0,
                    )
                else:
                    nc.scalar.activation(
                        f_tile, tp, Exp, bias=biasmu_sb, scale=1.0,
                    )
                for s in range(16):
                    t = 16 * g + s
                    k, sub = s // 4, s % 4
                    r = 32 * sub
                    c = 128 * k
                    if t == 0:
                        pa = f_tile[0:ST, 0:WA]
                        pb = f_tile[0:ST, WA:BC]
                        continue
                    step(
                        f_tile[r : r + ST, c : c + WA],
                        f_tile[r : r + ST, c + WA : c + BC],
                    )

            # virtual step 512: capture len==512 columns via fin
            step(fin_sb[:, 0:WA], fin_sb[:, WA:BC])

            # fs = omega row = esel^T state
            fs_ps = qa_pool.tile([1, BC], f32, tag="fsps", bufs=1)
            nc.tensor.matmul(fs_ps[:, 0:WA], esel_sb, pa, start=True, stop=True)
            nc.tensor.matmul(fs_ps[:, WA:BC], esel_sb, pb, start=True, stop=True)
            nc.vector.tensor_copy(fs_sb, fs_ps)
            nc.sync.dma_start(out=fs_d[:, :], in_=fs_sb)

    if not nc.is_finalized():
        nc.finalize()
    return nc


def _gold_score(feats, labels, lengths, trans):
    pos = np.arange(S)[None, :]
    valid = pos < lengths[:, None]
    emit = np.take_along_axis(feats, labels[:, :, None], axis=2)[:, :, 0]
    emit_sum = np.where(valid, emit, 0.0).sum(axis=1)
    start_sc = trans[START, labels[:, 0]]
    pair = trans[labels[:, :-1], labels[:, 1:]]
    pair_sum = np.where(valid[:, 1:], pair, 0.0).sum(axis=1)
    last_tag = np.take_along_axis(labels, (lengths - 1)[:, None], axis=1)[:, 0]
    stop_sc = trans[last_tag, STOP]
    return emit_sum + start_sc + pair_sum + stop_sc


def kernel(feats, labels, lengths, transitions):
    global _NC_CACHE, _LAST_RESULTS
    import ml_dtypes
    from concourse.bass_utils import run_bass_kernel_spmd

    feats = np.asarray(feats, dtype=np.float32)
    labels = np.asarray(labels, dtype=np.int64)
    lengths = np.asarray(lengths, dtype=np.int64)
    trans = np.asarray(transitions, dtype=np.float32)

    if _NC_CACHE is None:
        _NC_CACHE = _build_bass()
    nc = _NC_CACHE

    # augmented feats: [B, S, 32] = [feats[kept] | theta_log]
    aug = np.empty((B, S, ST), np.float32)
    aug[:, :, :31] = feats[:, :, KEPT]
    rows = lengths <= S - 1
    aug[np.arange(B)[rows], lengths[rows], :31] = NEG  # kill column at t==len
    aug[:, :, 31] = np.where(
        np.arange(S)[None, :] >= lengths[:, None], MU, NEG
    )  # theta step fn (0 at t=0 since len>=1)

    w = np.zeros((ST, ST), np.float32)
    w[:31, :31] = np.exp(trans)[np.ix_(KEPT, KEPT)]
    w[:31, 31] = 1.0  # omega' = colsum(P) + omega
    w[31, 31] = 1.0
    w = w.astype(ml_dtypes.bfloat16)

    bias0 = np.full((128, 1), -MU, np.float32)
    bias0[0:31, 0] = trans[START, KEPT] - MU

    ident = np.eye(128, dtype=np.float32)

    fin_full = np.zeros((ST, B), ml_dtypes.bfloat16)
    fin_full[31] = 1.0  # virtual step 512: theta=1 everywhere, tags die

    esel = np.zeros((ST, 1), ml_dtypes.bfloat16)
    esel[31, 0] = 1.0

    in_maps = []
    for c in range(NCORES):
        sl = slice(c * BC, (c + 1) * BC)
        in_maps.append(
            {
                "aug": aug[sl],
                "w": w,
                "bias0": bias0,
                "fin": np.ascontiguousarray(fin_full[:, sl]),
                "esel": esel,
                "ident": ident,
            }
        )

    trace = bool(int(os.environ.get("BASS_KERNEL_TRACE", "0")))
    kw = {}
    if trace:
        import concourse.bass_utils as _bu

        _install_ntff_hook()
        _bu.upload_artifacts = lambda tmpdir: "local://" + tmpdir
        import tempfile

        root = os.environ.get("BASS_TRACE_DIR", "/tmp/bass_trace")
        os.makedirs(root, exist_ok=True)
        tdir = tempfile.mkdtemp(dir=root)
        kw = {"tmpdir": tdir}
    res = run_bass_kernel_spmd(nc, in_maps, list(range(NCORES)), trace=trace, **kw)
    _LAST_RESULTS = res

    fs = np.concatenate([res.results[c]["fs"][0] for c in range(NCORES)])  # [B]
    forward = np.log(fs.astype(np.float64)) + lengths * MU
    gold = _gold_score(feats, labels, lengths, trans).astype(np.float64)
    loss = np.sum(forward - gold) / B
    return np.asarray(loss, dtype=np.float32)


# revision 20
# speedup vs baseline: 5990.1333x; 1.0055x over previous
"""BERT-CRF loss kernel for Trainium2 (8 NeuronCores, Bass/Tile).

Scaled-exp domain CRF forward with an exactly-32-row state per batch column:
  rows 0-30: P = exp(alpha[kept_tags] - t*MU)   (START tag dropped: provably 0)
  row 31:    omega = captured forward value (self-holding)

One step (t = 1..511):  state' = (W^T state) * F_t   where
  W[:31, :31] = exp(trans)[kept, kept]   (CRF transition mixing)
  W[:31, 31]  = 1, W[31, 31] = 1         (omega' = colsum(P) + omega)
  F_t rows 0-30 = exp(feat_t - MU)  (0 at the death step t==len: host scatter)
  F_t row 31    = theta_t = (t >= len)   (step fn; omega captures once because
                                          P dies at the death step, so colsum
                                          contributes only at t==len)

K=M=32 keeps every matmul in a single PE row/col group (one instruction), and
the 32-channel augmented feats keep all SBUF partition slices 32-aligned.
After a final virtual step 512 (captures len==512), forward[b] =
log(omega) + len[b]*MU. Gold score is pure gathers, done on host.
Validated: max |log| magnitude ~59 < 88 (fp32 safe) with MU=4.3.
"""

import os
import sys

import numpy as np

NUM_TAGS = 32
START = 30
STOP = 31
B = 1024
S = 512
NCORES = 8
BC = B // NCORES  # 128 batch per core
MU = 4.3
ST = 32  # state rows: 31 kept tags + omega
WA = 64  # chain A (DVE) batch columns
WB = BC - WA  # chain B (ACT) batch columns
NEG = -1.0e9
KEPT = list(range(30)) + [31]  # all tags except START

for _p in ("/opt/trn_rl_repo", "/root/.axon_site/_ro/trn_rl_repo"):
    if os.path.isdir(_p) and _p not in sys.path:
        sys.path.append(_p)

_NC_CACHE = None
_LAST_RESULTS = None  # BassKernelResults of most recent device run (for test.py)


def _install_ntff_hook():
    """Shim antenv.axon_hooks (absent in this image) so trace=True works."""
    import types

    if "antenv.axon_hooks" in sys.modules:
        return
    mod = types.ModuleType("antenv.axon_hooks")
    mod._hook = None
    mod.set_axon_ntff_profile_hook = lambda h: setattr(mod, "_hook", h)
    mod.get_axon_ntff_profile_hook = lambda: mod._hook
    sys.modules["antenv.axon_hooks"] = mod
    try:
        import antenv

        antenv.axon_hooks = mod
    except ImportError:
        pass
    try:
        from trn_agent_boot.trn_boot import _ntff_profile_via_ctypes

        h = _ntff_profile_via_ctypes("/opt/axon/libaxon_pjrt.so")
        if h is not None:
            mod._hook = h
    except Exception:
        pass


def _build_bass():
    import concourse.bacc as bacc
    import concourse.tile as tile
    from concourse import mybir

    f32 = mybir.dt.float32
    bf16 = mybir.dt.bfloat16
    nc = bacc.Bacc(None)

    aug_d = nc.declare_dram_parameter("aug", [BC, S, ST], f32, isOutput=False)
    w_d = nc.declare_dram_parameter("w", [2 * ST, ST], bf16, isOutput=False)
    bias0_d = nc.declare_dram_parameter("bias0", [128, 1], f32, isOutput=False)
    fin_d = nc.declare_dram_parameter("fin", [ST, BC], bf16, isOutput=False)
    esel_d = nc.declare_dram_parameter("esel", [2 * ST, 1], bf16, isOutput=False)
    ident_d = nc.declare_dram_parameter("ident", [128, 128], f32, isOutput=False)
    fs_d = nc.declare_dram_parameter("fs", [1, BC], f32, isOutput=True)

    Exp = mybir.ActivationFunctionType.Exp

    with tile.TileContext(nc) as tc:
        with (
            tc.tile_pool(name="const", bufs=1) as const,
            tc.tile_pool(name="fsb", bufs=3) as fsb_pool,
            tc.tile_pool(name="fexp", bufs=3) as f_pool,
            tc.tile_pool(name="pa", bufs=2) as pa_pool,
            tc.tile_pool(name="pb", bufs=2) as pb_pool,
            tc.tile_pool(name="tp", bufs=2, space="PSUM") as tp_pool,
            tc.tile_pool(name="qa", bufs=1, space="PSUM") as qa_pool,
            tc.tile_pool(name="qb", bufs=1, space="PSUM") as qb_pool,
        ):
            w_sb = const.tile([2 * ST, ST], bf16)
            nc.sync.dma_start(out=w_sb, in_=w_d[:, :])
            bias0_sb = const.tile([128, 1], f32)
            nc.sync.dma_start(out=bias0_sb, in_=bias0_d[:, :])
            ident_sb = const.tile([128, 128], f32)
            nc.sync.dma_start(out=ident_sb, in_=ident_d[:, :])
            fin_sb = const.tile([ST, BC], bf16)
            nc.sync.dma_start(out=fin_sb, in_=fin_d[:, :])
            esel_sb = const.tile([2 * ST, 1], bf16)
            nc.sync.dma_start(out=esel_sb, in_=esel_d[:, :])
            biasmu_sb = const.tile([128, 1], f32)
            nc.vector.memset(biasmu_sb, -MU)
            fs_sb = const.tile([1, BC], f32)

            # pre-touch DMA'd constants on PE so real PE ops carry <=1 new wait
            warm = qa_pool.tile([ST, BC], f32, tag="qa")
            nc.tensor.transpose(warm[0:ST, :], ident_sb[:, 0:ST], ident_sb)
            nc.tensor.matmul(
                warm[:, 0:ST], w_sb[0:ST, :], w_sb[0:ST, 0:ST],
                start=True, stop=True,
            )
            nc.tensor.matmul(warm, w_sb[0:ST, :], fin_sb, start=True, stop=True)
            nc.tensor.matmul(
                warm[:, 0:1], w_sb[0:ST, :], esel_sb[0:ST, :], start=True, stop=True
            )

            # chain A state lives at partition base 0 (PE row group 0);
            # chain B at partition base 32 (row group 1) so the two per-step
            # matmuls run concurrently in different PE sub-arrays.
            pa = None  # [32, WA] AP, base 0
            pb = None  # [32, WB] AP, base 0 (t=1 init) or 32
            pb_base = 0

            def step(f_ap_a, f_ap_b):
                """state' = (W^T state) * F; per chain: matmul + elementwise."""
                nonlocal pa, pb, pb_base
                qa = qa_pool.tile([ST, WA], f32, tag="qa")
                nc.tensor.matmul(qa, w_sb[0:ST, :], pa, start=True, stop=True)
                pa_n = pa_pool.tile([ST, WA], bf16, tag="pa")
                nc.vector.tensor_mul(pa_n, qa, f_ap_a)
                qb = qb_pool.tile([ST, WB], f32, tag="qb")
                nc.tensor.matmul(
                    qb, w_sb[pb_base : pb_base + ST, :], pb, start=True, stop=True
                )
                pb_t = pb_pool.tile([2 * ST, WB], bf16, tag="pb")
                pb_n = pb_t[ST : 2 * ST, :]
                nc.vector.tensor_mul(pb_n, qb, f_ap_b)
                pa, pb = pa_n, pb_n
                pb_base = ST

            for g in range(S // 16):  # 16 steps per staging group
                fsb = fsb_pool.tile([128, 16 * ST], f32, tag="fsb")
                nc.sync.dma_start(
                    out=fsb,
                    in_=aug_d[:, 16 * g : 16 * (g + 1), :].rearrange(
                        "b t j -> b (t j)"
                    ),
                )
                tp = tp_pool.tile([128, 512], f32, tag="tp")
                for k in range(4):  # 4 steps per 128x128 transpose
                    nc.tensor.transpose(
                        tp[:, 128 * k : 128 * (k + 1)],
                        fsb[:, 128 * k : 128 * (k + 1)],
                        ident_sb,
                    )
                f_tile = f_pool.tile([128, 512], bf16, tag="f")
                if g == 0:
                    # col-block 0 holds t=0..3; its partition rows 0-31 are
                    # t=0, which get the START-transition bias.
                    nc.scalar.activation(
                        f_tile[:, 0:128], tp[:, 0:128], Exp,
                        bias=bias0_sb, scale=1.0,
                    )
                    nc.scalar.activation(
                        f_tile[:, 128:512], tp[:, 128:512], Exp,
                        bias=biasmu_sb, scale=1.0,
                    )
                else:
                    nc.scalar.activation(
                        f_tile, tp, Exp, bias=biasmu_sb, scale=1.0,
                    )
                for s in range(16):
                    t = 16 * g + s
                    k, sub = s // 4, s % 4
                    r = 32 * sub
                    c = 128 * k
                    if t == 0:
                        pa = f_tile[0:ST, 0:WA]
                        pb = f_tile[0:ST, WA:BC]
                        continue
                    step(
                        f_tile[r : r + ST, c : c + WA],
                        f_tile[r : r + ST, c + WA : c + BC],
                    )

            # virtual step 512: capture len==512 columns via fin
            step(fin_sb[:, 0:WA], fin_sb[:, WA:BC])

            # fs = omega row = esel^T state
            fs_ps = qa_pool.tile([1, BC], f32, tag="fsps", bufs=1)
            nc.tensor.matmul(
                fs_ps[:, 0:WA], esel_sb[0:ST, :], pa, start=True, stop=True
            )
            nc.tensor.matmul(
                fs_ps[:, WA:BC], esel_sb[ST : 2 * ST, :], pb, start=True, stop=True
            )
            nc.vector.tensor_copy(fs_sb, fs_ps)
            nc.sync.dma_start(out=fs_d[:, :], in_=fs_sb)

    if not nc.is_finalized():
        nc.finalize()
    return nc


def _gold_score(feats, labels, lengths, trans):
    pos = np.arange(S)[None, :]
    valid = pos < lengths[:, None]
    emit = np.take_along_axis(feats, labels[:, :, None], axis=2)[:, :, 0]
    emit_sum = np.where(valid, emit, 0.0).sum(axis=1)
    start_sc = trans[START, labels[:, 0]]
    pair = trans[labels[:, :-1], labels[:, 1:]]
    pair_sum = np.where(valid[:, 1:], pair, 0.0).sum(axis=1)
    last_tag = np.take_along_axis(labels, (lengths - 1)[:, None], axis=1)[:, 0]
    stop_sc = trans[last_tag, STOP]
    return emit_sum + start_sc + pair_sum + stop_sc


def kernel(feats, labels, lengths, transitions):
    global _NC_CACHE, _LAST_RESULTS
    import ml_dtypes
    from concourse.bass_utils import run_bass_kernel_spmd

    feats = np.asarray(feats, dtype=np.float32)
    labels = np.asarray(labels, dtype=np.int64)
    lengths = np.asarray(lengths, dtype=np.int64)
    trans = np.asarray(transitions, dtype=np.float32)

    if _NC_CACHE is None:
        _NC_CACHE = _build_bass()
    nc = _NC_CACHE

    # augmented feats: [B, S, 32] = [feats[kept] | theta_log]
    aug = np.empty((B, S, ST), np.float32)
    aug[:, :, :31] = feats[:, :, KEPT]
    rows = lengths <= S - 1
    aug[np.arange(B)[rows], lengths[rows], :31] = NEG  # kill column at t==len
    aug[:, :, 31] = np.where(
        np.arange(S)[None, :] >= lengths[:, None], MU, NEG
    )  # theta step fn (0 at t=0 since len>=1)

    w = np.zeros((ST, ST), np.float32)
    w[:31, :31] = np.exp(trans)[np.ix_(KEPT, KEPT)]
    w[:31, 31] = 1.0  # omega' = colsum(P) + omega
    w[31, 31] = 1.0
    w = np.concatenate([w, w], axis=0).astype(ml_dtypes.bfloat16)

    bias0 = np.full((128, 1), -MU, np.float32)
    bias0[0:31, 0] = trans[START, KEPT] - MU

    ident = np.eye(128, dtype=np.float32)

    fin_full = np.zeros((ST, B), ml_dtypes.bfloat16)
    fin_full[31] = 1.0  # virtual step 512: theta=1 everywhere, tags die

    esel = np.zeros((2 * ST, 1), ml_dtypes.bfloat16)
    esel[31, 0] = 1.0
    esel[63, 0] = 1.0

    in_maps = []
    for c in range(NCORES):
        sl = slice(c * BC, (c + 1) * BC)
        in_maps.append(
            {
                "aug": aug[sl],
                "w": w,
                "bias0": bias0,
                "fin": np.ascontiguousarray(fin_full[:, sl]),
                "esel": esel,
                "ident": ident,
            }
        )

    trace = bool(int(os.environ.get("BASS_KERNEL_TRACE", "0")))
    kw = {}
    if trace:
        import concourse.bass_utils as _bu

        _install_ntff_hook()
        _bu.upload_artifacts = lambda tmpdir: "local://" + tmpdir
        import tempfile

        root = os.environ.get("BASS_TRACE_DIR", "/tmp/bass_trace")
        os.makedirs(root, exist_ok=True)
        tdir = tempfile.mkdtemp(dir=root)
        kw = {"tmpdir": tdir}
    res = run_bass_kernel_spmd(nc, in_maps, list(range(NCORES)), trace=trace, **kw)
    _LAST_RESULTS = res

    fs = np.concatenate([res.results[c]["fs"][0] for c in range(NCORES)])  # [B]
    forward = np.log(fs.astype(np.float64)) + lengths * MU
    gold = _gold_score(feats, labels, lengths, trans).astype(np.float64)
    loss = np.sum(forward - gold) / B
    return np.asarray(loss, dtype=np.float32)
